# revision 14
# baseline (speedup 1.0000x reference)
"""Causal single-head attention (B=4, T=4096, C=1024, H=64) on 8 trn2 NeuronCores.

Sharding: core = (batch b = core//2, parity p = core%2). Each core owns the
interleaved context tiles {p, p+2, ...} of its batch (balanced under the causal
mask) and computes partial flash-attention (numerator + denominator) for ALL
queries of the batch; the host sums the two partials per batch and divides.

Host-side prep is layout only: the x shard is passed pre-transposed [C, T/2]
(pure permutation), weights pre-fused into their SBUF layout, masks as data.
All numerics (projections, scores, softmax, PV) run on device.

Device pipeline per core:
  load x^T -> project [K^|Q^] with a fused [wk|wq] stationary (fp32r) ->
  pairwise AllGather of the bf16 k-projection in 4 pipelined chunks (scores
  use S=k, G=q per the reference's K@Q^T convention) -> per 512-query block:
  row-packed bf16 score matmuls -> exp on ACT (scale folded in) -> data-driven
  causal masks on DVE -> bf16 PV matmuls accumulating [V|1]^T @ P^T in dual
  PSUM chains -> write O^T_aug [H+1, T].

Query columns are processed in by-rank order (rank0 tiles | rank1 tiles per
512-block); the host maps them back to absolute order in combine_outputs.
"""

import sys

for _p in ("/root/.axon_site/_ro/trn_rl_repo", "/root/.axon_site/_ro/pypackages"):
    if _p not in sys.path:
        sys.path.append(_p)

import ml_dtypes
import numpy as np

import concourse.bass as bass
import concourse.mybir as mybir
import concourse.tile as tile
from concourse import bacc
from concourse.bass_utils import run_bass_kernel_spmd
from concourse.masks import make_identity

B, T, C, H = 4, 4096, 1024, 64
N_CORES = 8
SCALE = C ** -0.5
F32 = mybir.dt.float32
F32R = mybir.dt.float32r
BF16 = mybir.dt.bfloat16
REPLICA_GROUPS = [[0, 1], [2, 3], [4, 5], [6, 7]]
SUB2ABS = [0, 2, 1, 3]      # 128-col sub-tile -> abs tile offset within 512-blk


def build_kernel(t_full=T):
    """Build the SPMD Bass/Tile program for sequence length t_full."""
    t_own = t_full // 2           # context rows owned by this core
    n_own = t_own // 128          # own 128-row s-tiles
    n_blk = t_full // 512         # 512-wide query blocks
    n_cchunk = C // 128           # contraction chunks of 128
    n_tq = t_own // 512           # projection column blocks == gather chunks

    nc = bacc.Bacc("TRN2", target_bir_lowering=False, debug=False,
                   num_devices=N_CORES)

    xt_d = nc.dram_tensor("xt_own", [C, t_own], F32, kind="ExternalInput").ap()
    wkq_d = nc.dram_tensor("wkq", [128, n_cchunk * 128], F32,
                           kind="ExternalInput").ap()
    wv_d = nc.dram_tensor("wv", [128, n_cchunk * H], F32,
                          kind="ExternalInput").ap()
    bkq_d = nc.dram_tensor("bkq", [128], F32, kind="ExternalInput").ap()
    bv_d = nc.dram_tensor("bv", [64], F32, kind="ExternalInput").ap()
    masks_d = nc.dram_tensor("masks", [2, 128, 512], BF16,
                             kind="ExternalInput").ap()
    out_d = nc.dram_tensor("out_part", [H + 1, t_full], F32,
                           kind="ExternalOutput").ap()

    with tile.TileContext(nc) as tc:
        with (
            tc.tile_pool(name="persist", bufs=1) as pp,
            tc.tile_pool(name="dram", bufs=1, space="DRAM") as dp,
            tc.tile_pool(name="psB", bufs=2, space="PSUM") as psb,
            tc.tile_pool(name="psS", bufs=2, space="PSUM") as pss,
            tc.tile_pool(name="psOA", bufs=1, space="PSUM") as psoa,
            tc.tile_pool(name="psOB", bufs=1, space="PSUM") as psob,
            tc.tile_pool(name="ptp", bufs=4) as ptp,
            tc.tile_pool(name="outp", bufs=2) as outp,
        ):
            # ---- persistent SBUF tensors ----
            xt = pp.tile([128, n_cchunk * t_own], F32R)   # x^T, chunk j at cols t_own*j
            kqT = pp.tile([128, t_own], BF16)             # 0:64 = S^T own, 64:128 = G^T own
            gt_lo = pp.tile([64, t_own], BF16)            # G^T copy at partitions 0:64
            stf = pp.tile([128, t_full], BF16)            # S^T by rank, dup row halves
            vT = pp.tile([64, t_own], F32)                # V^T own
            v_sb = pp.tile([128, n_own * 65], BF16)       # V_aug tiles [128,65]
            wkq_sb = pp.tile([128, n_cchunk * 128], F32R)  # [wk|wq] fused stationary
            wv_sb = pp.tile([128, n_cchunk * H], F32R)
            bias_kq = pp.tile([128, 1], F32)
            bias_v = pp.tile([64, 1], F32)
            mask0 = pp.tile([128, 512], BF16)
            mask1 = pp.tile([128, 512], BF16)
            ident = pp.tile([128, 128], F32)

            make_identity(nc, ident[:, :])

            # x^T loads: one DMA per 512-col tq slice covering all c-chunks,
            # so projection block tq depends only on slice tq
            xt_v = xt[:, :].rearrange("p (j t) -> p j t", t=t_own)
            xts_v = xt_d.rearrange("(j p) t -> p j t", p=128)
            for tq in range(n_tq):
                nc.sync.dma_start(
                    out=xt_v[:, :, 512 * tq:512 * (tq + 1)],
                    in_=xts_v[:, :, 512 * tq:512 * (tq + 1)].bitcast(F32R))

            # small operands on the scalar HWDGE queue (single contiguous DMAs)
            nc.scalar.dma_start(out=wkq_sb[:, :], in_=wkq_d.bitcast(F32R))
            nc.scalar.dma_start(out=wv_sb[:, :], in_=wv_d.bitcast(F32R))
            nc.scalar.dma_start(out=bias_kq[:, 0:1], in_=bkq_d[:, None])
            nc.scalar.dma_start(out=bias_v[:, 0:1], in_=bv_d[:, None])
            nc.scalar.dma_start(out=mask0[:, :], in_=masks_d[0])
            nc.scalar.dma_start(out=mask1[:, :], in_=masks_d[1])

            # ones column of V_aug (col 64 of each 65-wide slot)
            nc.vector.tensor_scalar(
                v_sb[:, :].rearrange("p (i c) -> p i c", c=65)[:, :, 64],
                ident[:, 0:n_own], 0.0, 1.0,
                op0=mybir.AluOpType.mult, op1=mybir.AluOpType.add)

            # tiny warmup collective: absorbs TOPSP/mesh startup latency while
            # the x stream loads; sourced from ident (ready at preamble)
            ccw_i = dp.tile([64, 16], F32, name="ccwi", tag="ccwi")
            ccw_o = dp.tile([128, 16], F32, name="ccwo", tag="ccwo")
            nc.gpsimd.dma_start(ccw_i[:, :], ident[0:64, 0:16])
            nc.gpsimd.collective_compute(
                "AllGather", mybir.AluOpType.bypass,
                replica_groups=REPLICA_GROUPS,
                ins=[ccw_i[:, :].opt()],
                outs=[ccw_o[:, :].opt()],
            )

            cc_in = [dp.tile([64, 512], BF16, name=f"cci{c}", tag=f"cci{c}")
                     for c in range(n_tq)]
            cc_out = [dp.tile([128, 512], BF16, name=f"cco{c}", tag=f"cco{c}")
                      for c in range(n_tq)]

            def project_kq(tq):
                ps = psb.tile([128, 512], F32, tag="psB")
                rhs = xt_v[:, :, 512 * tq:512 * (tq + 1)]
                for j in range(n_cchunk):
                    nc.tensor.matmul(
                        ps[:, :], wkq_sb[:, 128 * j:128 * (j + 1)],
                        rhs[:, j], start=(j == 0), stop=(j == n_cchunk - 1))
                nc.vector.tensor_scalar_add(
                    kqT[:, 512 * tq:512 * (tq + 1)], ps[:, :], bias_kq[:, 0:1])

            def project_v(tq):
                ps = psb.tile([64, 512], F32, tag="psB")
                rhs = xt_v[:, :, 512 * tq:512 * (tq + 1)]
                for j in range(n_cchunk):
                    nc.tensor.matmul(
                        ps[:, :], wv_sb[:, H * j:H * (j + 1)],
                        rhs[:, j], start=(j == 0), stop=(j == n_cchunk - 1))
                nc.vector.tensor_scalar_add(
                    vT[:, 512 * tq:512 * (tq + 1)], ps[:, :], bias_v[:, 0:1])

            def v_transpose(i):
                ps = psb.tile([128, 64], F32, tag="psB")
                nc.tensor.transpose(
                    ps[:, :], vT[:, 128 * i:128 * (i + 1)], ident[0:64, 0:64])
                nc.vector.tensor_copy(v_sb[:, 65 * i:65 * i + 64], ps[:, :])

            def gather_chunk(ch):
                nc.gpsimd.dma_start(cc_in[ch][:, :],
                                    kqT[0:64, 512 * ch:512 * (ch + 1)])
                nc.gpsimd.collective_compute(
                    "AllGather", mybir.AluOpType.bypass,
                    replica_groups=REPLICA_GROUPS,
                    ins=[cc_in[ch][:, :].opt()],
                    outs=[cc_out[ch][:, :].opt()],
                )
                # stf: [rank0 t_own | rank1 t_own], both partition halves
                for q in (0, 1):
                    for dh in (0, 1):
                        nc.sync.dma_start(
                            out=stf[64 * dh:64 * (dh + 1),
                                    t_own * q + 512 * ch:
                                    t_own * q + 512 * (ch + 1)],
                            in_=cc_out[ch][64 * q:64 * (q + 1), :])
                # G^T duplicate into partitions 0:64
                nc.sync.dma_start(gt_lo[:, 512 * ch:512 * (ch + 1)],
                                  kqT[64:128, 512 * ch:512 * (ch + 1)])

            # ---- projections + pipelined gathers ----
            for tq in range(n_tq):
                project_kq(tq)
                gather_chunk(tq)
                project_v(tq)
                for i in range(4 * tq, min(4 * (tq + 1), n_own)):
                    v_transpose(i)

            # ---- flash attention main loop ----
            stf_lo = stf[0:64, :].rearrange("p (h t) -> p h t", h=2)
            stf_hi = stf[64:128, :].rearrange("p (h t) -> p h t", h=2)
            for tb in range(n_blk):
                poa = psoa.tile([65, 512], F32, tag="psOA")
                pob = psob.tile([65, 512], F32, tag="psOB")
                for ip in range(tb + 1):
                    i0, i1 = 2 * ip, 2 * ip + 1
                    ps = pss.tile([128, 1024], F32, tag="psS")
                    pt = ptp.tile([128, 1024], BF16, tag="pt")
                    nc.tensor.matmul(
                        ps[:, 0:512],
                        gt_lo[:, 128 * i0:128 * (i0 + 1)],
                        stf_lo[:, :, 256 * tb:256 * (tb + 1)],
                        start=True, stop=True, tile_position=(0, 0))
                    nc.tensor.matmul(
                        ps[:, 512:1024],
                        kqT[64:128, 128 * i1:128 * (i1 + 1)],
                        stf_hi[:, :, 256 * tb:256 * (tb + 1)],
                        start=True, stop=True, tile_position=(64, 0))
                    nc.scalar.activation(
                        pt[:, :], ps[:, :],
                        mybir.ActivationFunctionType.Exp, scale=SCALE)
                    if ip == tb:
                        nc.vector.tensor_mul(
                            pt[:, 0:512], pt[:, 0:512], mask0[:, :])
                        nc.vector.tensor_mul(
                            pt[:, 512:1024], pt[:, 512:1024], mask1[:, :])
                    nc.tensor.matmul(
                        poa[:, :], v_sb[:, 65 * i0:65 * (i0 + 1)],
                        pt[:, 0:512], start=(ip == 0), stop=(ip == tb))
                    nc.tensor.matmul(
                        pob[:, :], v_sb[:, 65 * i1:65 * (i1 + 1)],
                        pt[:, 512:1024], start=(ip == 0), stop=(ip == tb))
                ob = outp.tile([65, 512], F32, tag="ob")
                nc.vector.tensor_copy(ob[:, :], poa[:, :])
                nc.vector.tensor_add(ob[:, :], pob[:, :], ob[:, :])
                nc.sync.dma_start(
                    out=out_d[:, 512 * tb:512 * (tb + 1)], in_=ob[:, :])

    nc.compile()
    return nc


def make_core_inputs(x, Wk, bk, Wq, bq, Wv, bv, t_full=T):
    """Shard FULL inputs into the 8 per-core input dicts (layout prep only)."""
    n_tiles = t_full // 128
    n_cchunk = C // 128
    Wk = np.asarray(Wk, np.float32)
    Wq = np.asarray(Wq, np.float32)
    Wv = np.asarray(Wv, np.float32)
    # fused [wk|wq] stationary: wkq[p, 128j + 64s + h] = (Wk,Wq)[s][128j+p, h]
    wkq = np.empty((128, n_cchunk * 128), np.float32)
    wvf = np.empty((128, n_cchunk * H), np.float32)
    for j in range(n_cchunk):
        wkq[:, 128 * j:128 * j + 64] = Wk[128 * j:128 * (j + 1), :]
        wkq[:, 128 * j + 64:128 * (j + 1)] = Wq[128 * j:128 * (j + 1), :]
        wvf[:, H * j:H * (j + 1)] = Wv[128 * j:128 * (j + 1), :]
    bkq = np.concatenate([np.asarray(bk, np.float32),
                          np.asarray(bq, np.float32)])
    ins = []
    for core in range(N_CORES):
        b, p = core // 2, core % 2
        own = np.concatenate(
            [x[b, 128 * j:128 * (j + 1), :] for j in range(p, n_tiles, 2)],
            axis=0)
        # mask[m][r, c]: s-tile (local parity m, abs tile 4tb+2m+p) vs query
        # sub-tile c//128 (abs tile 4tb + SUB2ABS[c//128]); valid iff s <= t
        masks = np.zeros((2, 128, 512), np.float32)
        rr = np.arange(128)[:, None]
        for m in (0, 1):
            for sub in range(4):
                cz = np.arange(128)[None, :]
                s_abs = 128 * (2 * m + p) + rr
                t_abs = 128 * SUB2ABS[sub] + cz
                masks[m, :, 128 * sub:128 * (sub + 1)] = (s_abs <= t_abs)
        ins.append({
            "xt_own": np.ascontiguousarray(own.T, dtype=np.float32),
            "wkq": wkq, "wv": wvf, "bkq": bkq,
            "bv": np.asarray(bv, np.float32),
            "masks": masks.astype(ml_dtypes.bfloat16),
        })
    return ins


def _col_perm(t_full):
    """stored column -> absolute t index (same for every core)."""
    perm = np.empty(t_full, np.int64)
    for tb in range(t_full // 512):
        for sub in range(4):
            a = 128 * (4 * tb + SUB2ABS[sub])
            s = 512 * tb + 128 * sub
            perm[s:s + 128] = np.arange(a, a + 128)
    return perm


def combine_outputs(parts, t_full=T):
    """parts: list of 8 arrays [H+1, t_full] -> full output [B, t_full, H]."""
    perm = _col_perm(t_full)
    out = np.empty((B, t_full, H), np.float32)
    for b in range(B):
        acc = parts[2 * b] + parts[2 * b + 1]
        res = acc[:H, :] / acc[H:H + 1, :]
        out[b][perm] = res.T
    return out


_NC_CACHE = {}


def kernel(x, Wk, bk, Wq, bq, Wv, bv):
    x = np.asarray(x, np.float32)
    t_full = x.shape[1]
    if t_full not in _NC_CACHE:
        _NC_CACHE[t_full] = build_kernel(t_full)
    nc = _NC_CACHE[t_full]
    ins = make_core_inputs(x, Wk, bk, Wq, bq, Wv, bv, t_full)
    res = run_bass_kernel_spmd(nc, ins, list(range(N_CORES)))
    parts = [res.results[i]["out_part"] for i in range(N_CORES)]
    return combine_outputs(parts, t_full)


if __name__ == "__main__":
    rng = np.random.default_rng(0)
    x = rng.standard_normal((B, T, C), dtype=np.float32)
    Wk = rng.standard_normal((C, H), dtype=np.float32) * SCALE
    Wq = rng.standard_normal((C, H), dtype=np.float32) * SCALE
    Wv = rng.standard_normal((C, H), dtype=np.float32) * SCALE
    bk = rng.standard_normal(H).astype(np.float32) * 0.02
    bq = rng.standard_normal(H).astype(np.float32) * 0.02
    bv = rng.standard_normal(H).astype(np.float32) * 0.02
    out = kernel(x=x, Wk=Wk, bk=bk, Wq=Wq, bq=bq, Wv=Wv, bv=bv)
    print(out.shape, out.dtype)


# revision 15
# speedup vs baseline: 1.2304x; 1.2304x over previous
"""Causal single-head attention (B=4, T=4096, C=1024, H=64) on 8 trn2 NeuronCores.

Sharding: core = (batch b = core//2, parity p = core%2). Each core owns the
interleaved context tiles {p, p+2, ...} of its batch (balanced under the causal
mask) and computes partial flash-attention (numerator + denominator) for ALL
queries of the batch; the host sums the two partials per batch and divides.

Host-side prep is layout only: the x shard is passed pre-transposed [C, T]
with columns ordered [own tiles | partner tiles] (pure permutation), weights
pre-fused into their SBUF layout, causal masks as data. All numerics run on
device. No collectives: each core projects the k-side for the full sequence
locally (trades HBM reads for the multi-10us cold-start of on-chip
collectives).

Device pipeline per core:
  load x^T slices (interleaved own/partner order) -> project [S^|G^] with a
  fused [wk|wq] fp32r stationary as slices land -> per 512-query block:
  row-packed bf16 score matmuls -> exp on ACT (scale folded in) -> data-driven
  causal masks on DVE -> bf16 PV matmuls (V padded to 128 cols for fast
  weight load) accumulating [V|1]^T @ P^T in dual PSUM chains -> write
  O^T_aug [H+1, T].

Query columns are processed in [own|partner] order per 512-block; the host
maps them back to absolute order per core in combine_outputs.
"""

import sys

for _p in ("/root/.axon_site/_ro/trn_rl_repo", "/root/.axon_site/_ro/pypackages"):
    if _p not in sys.path:
        sys.path.append(_p)

import ml_dtypes
import numpy as np

import concourse.bass as bass
import concourse.mybir as mybir
import concourse.tile as tile
from concourse import bacc
from concourse.bass_utils import run_bass_kernel_spmd
from concourse.masks import make_identity

B, T, C, H = 4, 4096, 1024, 64
N_CORES = 8
SCALE = C ** -0.5
F32 = mybir.dt.float32
F32R = mybir.dt.float32r
BF16 = mybir.dt.bfloat16


def build_kernel(t_full=T):
    """Build the SPMD Bass/Tile program for sequence length t_full."""
    t_own = t_full // 2           # context rows owned by this core
    n_own = t_own // 128          # own 128-row s-tiles
    n_blk = t_full // 512         # 512-wide query blocks
    n_cchunk = C // 128           # contraction chunks of 128
    n_tq = t_full // 512          # projection/load slices (stored order)
    n_vq = t_own // 512           # V projection slices (own region)

    nc = bacc.Bacc("TRN2", target_bir_lowering=False, debug=False,
                   num_devices=N_CORES)

    xt_d = nc.dram_tensor("xt_own", [C, t_full], F32, kind="ExternalInput").ap()
    wkq_d = nc.dram_tensor("wkq", [128, n_cchunk * 128], F32,
                           kind="ExternalInput").ap()
    wv_d = nc.dram_tensor("wv", [128, n_cchunk * H], F32,
                          kind="ExternalInput").ap()
    bkq_d = nc.dram_tensor("bkq", [128], F32, kind="ExternalInput").ap()
    bv_d = nc.dram_tensor("bv", [64], F32, kind="ExternalInput").ap()
    masks_d = nc.dram_tensor("masks", [2, 128, 512], BF16,
                             kind="ExternalInput").ap()
    out_d = nc.dram_tensor("out_part", [H + 1, t_full], F32,
                           kind="ExternalOutput").ap()

    with tile.TileContext(nc) as tc:
        with (
            tc.tile_pool(name="persist", bufs=1) as pp,
            tc.tile_pool(name="psB", bufs=2, space="PSUM") as psb,
            tc.tile_pool(name="psS", bufs=2, space="PSUM") as pss,
            tc.tile_pool(name="psOA", bufs=1, space="PSUM") as psoa,
            tc.tile_pool(name="psOB", bufs=1, space="PSUM") as psob,
            tc.tile_pool(name="ptp", bufs=4) as ptp,
            tc.tile_pool(name="outp", bufs=2) as outp,
        ):
            # ---- persistent SBUF tensors ----
            xt = pp.tile([128, n_cchunk * t_full], F32R)  # x^T, chunk j at cols t_full*j
            kqT = pp.tile([128, t_full], BF16)            # 0:64 = S^T, 64:128 = G^T
            sd_hi = pp.tile([128, t_full], BF16)          # S^T dup at partitions 64:128
            gt_lo = pp.tile([64, t_own], BF16)            # G^T (own) at partitions 0:64
            vT = pp.tile([64, t_own], F32)                # V^T own
            v_sb = pp.tile([128, n_own * 128], BF16)      # V_aug tiles [128,128] (padded)
            wkq_sb = pp.tile([128, n_cchunk * 128], F32R)  # [wk|wq] fused stationary
            wv_sb = pp.tile([128, n_cchunk * H], F32R)
            bias_kq = pp.tile([128, 1], F32)
            bias_v = pp.tile([64, 1], F32)
            mask0 = pp.tile([128, 512], BF16)
            mask1 = pp.tile([128, 512], BF16)
            ident = pp.tile([128, 128], F32)

            make_identity(nc, ident[:, :])

            xt_v = xt[:, :].rearrange("p (j t) -> p j t", t=t_full)
            xts_v = xt_d.rearrange("(j p) t -> p j t", p=128)

            def load_slice(tq):
                nc.sync.dma_start(
                    out=xt_v[:, :, 512 * tq:512 * (tq + 1)],
                    in_=xts_v[:, :, 512 * tq:512 * (tq + 1)].bitcast(F32R))

            # small operands on the scalar HWDGE queue (single contiguous DMAs)
            nc.scalar.dma_start(out=wkq_sb[:, :], in_=wkq_d.bitcast(F32R))
            nc.scalar.dma_start(out=wv_sb[:, :], in_=wv_d.bitcast(F32R))
            nc.scalar.dma_start(out=bias_kq[:, 0:1], in_=bkq_d[:, None])
            nc.scalar.dma_start(out=bias_v[:, 0:1], in_=bv_d[:, None])
            nc.scalar.dma_start(out=mask0[:, :], in_=masks_d[0])
            nc.scalar.dma_start(out=mask1[:, :], in_=masks_d[1])

            # V_aug padding: zero cols 64:128 of each slot, ones at col 64
            nc.gpsimd.memset(v_sb[:, :], 0.0)
            nc.vector.tensor_scalar(
                v_sb[:, :].rearrange("p (i c) -> p i c", c=128)[:, :, 64],
                ident[:, 0:n_own], 0.0, 1.0,
                op0=mybir.AluOpType.mult, op1=mybir.AluOpType.add)

            def project_kq(tq):
                ps = psb.tile([128, 512], F32, tag="psB")
                rhs = xt_v[:, :, 512 * tq:512 * (tq + 1)]
                for j in range(n_cchunk):
                    nc.tensor.matmul(
                        ps[:, :], wkq_sb[:, 128 * j:128 * (j + 1)],
                        rhs[:, j], start=(j == 0), stop=(j == n_cchunk - 1))
                nc.vector.tensor_scalar_add(
                    kqT[:, 512 * tq:512 * (tq + 1)], ps[:, :], bias_kq[:, 0:1])
                # S^T duplicate into partitions 64:128 for row-packed scores
                nc.sync.dma_start(
                    sd_hi[64:128, 512 * tq:512 * (tq + 1)],
                    kqT[0:64, 512 * tq:512 * (tq + 1)])
                if tq < n_vq:
                    # G^T duplicate into partitions 0:64 (own region only)
                    nc.sync.dma_start(
                        gt_lo[:, 512 * tq:512 * (tq + 1)],
                        kqT[64:128, 512 * tq:512 * (tq + 1)])

            def project_v(tq):
                ps = psb.tile([64, 512], F32, tag="psB")
                rhs = xt_v[:, :, 512 * tq:512 * (tq + 1)]
                for j in range(n_cchunk):
                    nc.tensor.matmul(
                        ps[:, :], wv_sb[:, H * j:H * (j + 1)],
                        rhs[:, j], start=(j == 0), stop=(j == n_cchunk - 1))
                nc.vector.tensor_scalar_add(
                    vT[:, 512 * tq:512 * (tq + 1)], ps[:, :], bias_v[:, 0:1])

            def v_transpose(i):
                ps = psb.tile([128, 64], F32, tag="psB")
                nc.tensor.transpose(
                    ps[:, :], vT[:, 128 * i:128 * (i + 1)], ident[0:64, 0:64])
                nc.vector.tensor_copy(v_sb[:, 128 * i:128 * i + 64], ps[:, :])

            # ---- loads + projections, own/partner slices interleaved so
            # query block tb only needs slices {tb//2, n_vq + tb//2} ----
            order = []
            for k in range(n_vq):
                order += [k, n_vq + k]
            order += list(range(2 * n_vq, n_tq))  # (t_full==2*t_own: empty)
            for tq in order:
                load_slice(tq)
            for tq in order:
                project_kq(tq)
                if tq < n_vq:
                    project_v(tq)
                    for i in range(4 * tq, min(4 * (tq + 1), n_own)):
                        v_transpose(i)

            # ---- flash attention main loop ----
            kq_lo = kqT[0:64, :].rearrange("p (h t) -> p h t", h=2)
            sd_v = sd_hi[64:128, :].rearrange("p (h t) -> p h t", h=2)
            for tb in range(n_blk):
                poa = psoa.tile([128, 512], F32, tag="psOA")
                pob = psob.tile([128, 512], F32, tag="psOB")
                for ip in range(tb + 1):
                    i0, i1 = 2 * ip, 2 * ip + 1
                    ps = pss.tile([128, 1024], F32, tag="psS")
                    pt = ptp.tile([128, 1024], BF16, tag="pt")
                    nc.tensor.matmul(
                        ps[:, 0:512],
                        gt_lo[:, 128 * i0:128 * (i0 + 1)],
                        kq_lo[:, :, 256 * tb:256 * (tb + 1)],
                        start=True, stop=True, tile_position=(0, 0))
                    nc.tensor.matmul(
                        ps[:, 512:1024],
                        kqT[64:128, 128 * i1:128 * (i1 + 1)],
                        sd_v[:, :, 256 * tb:256 * (tb + 1)],
                        start=True, stop=True, tile_position=(64, 0))
                    nc.scalar.activation(
                        pt[:, :], ps[:, :],
                        mybir.ActivationFunctionType.Exp, scale=SCALE)
                    if ip == tb:
                        nc.vector.tensor_mul(
                            pt[:, 0:512], pt[:, 0:512], mask0[:, :])
                        nc.vector.tensor_mul(
                            pt[:, 512:1024], pt[:, 512:1024], mask1[:, :])
                    nc.tensor.matmul(
                        poa[:, :], v_sb[:, 128 * i0:128 * (i0 + 1)],
                        pt[:, 0:512], start=(ip == 0), stop=(ip == tb))
                    nc.tensor.matmul(
                        pob[:, :], v_sb[:, 128 * i1:128 * (i1 + 1)],
                        pt[:, 512:1024], start=(ip == 0), stop=(ip == tb))
                ob = outp.tile([65, 512], F32, tag="ob")
                nc.vector.tensor_copy(ob[:, :], poa[0:65, :])
                nc.vector.tensor_add(ob[:, :], pob[0:65, :], ob[:, :])
                nc.sync.dma_start(
                    out=out_d[:, 512 * tb:512 * (tb + 1)], in_=ob[:, :])

    nc.compile()
    return nc


def make_core_inputs(x, Wk, bk, Wq, bq, Wv, bv, t_full=T):
    """Shard FULL inputs into the 8 per-core input dicts (layout prep only)."""
    n_tiles = t_full // 128
    n_cchunk = C // 128
    Wk = np.asarray(Wk, np.float32)
    Wq = np.asarray(Wq, np.float32)
    Wv = np.asarray(Wv, np.float32)
    wkq = np.empty((128, n_cchunk * 128), np.float32)
    wvf = np.empty((128, n_cchunk * H), np.float32)
    for j in range(n_cchunk):
        wkq[:, 128 * j:128 * j + 64] = Wk[128 * j:128 * (j + 1), :]
        wkq[:, 128 * j + 64:128 * (j + 1)] = Wq[128 * j:128 * (j + 1), :]
        wvf[:, H * j:H * (j + 1)] = Wv[128 * j:128 * (j + 1), :]
    bkq = np.concatenate([np.asarray(bk, np.float32),
                          np.asarray(bq, np.float32)])
    ins = []
    for core in range(N_CORES):
        b, p = core // 2, core % 2
        own = np.concatenate(
            [x[b, 128 * j:128 * (j + 1), :] for j in range(p, n_tiles, 2)]
            + [x[b, 128 * j:128 * (j + 1), :]
               for j in range(1 - p, n_tiles, 2)],
            axis=0)
        # mask[m][r, c]: own s-tile (local parity m, abs tile 4tb+2m+p) vs
        # query sub-tile c//128 (abs tile 4tb + A[c//128]); valid iff s <= t
        A = [p, 2 + p, 1 - p, 3 - p]
        masks = np.zeros((2, 128, 512), np.float32)
        rr = np.arange(128)[:, None]
        for m in (0, 1):
            for sub in range(4):
                cz = np.arange(128)[None, :]
                s_abs = 128 * (2 * m + p) + rr
                t_abs = 128 * A[sub] + cz
                masks[m, :, 128 * sub:128 * (sub + 1)] = (s_abs <= t_abs)
        ins.append({
            "xt_own": np.ascontiguousarray(own.T, dtype=np.float32),
            "wkq": wkq, "wv": wvf, "bkq": bkq,
            "bv": np.asarray(bv, np.float32),
            "masks": masks.astype(ml_dtypes.bfloat16),
        })
    return ins


def _col_perm(p, t_full):
    """stored column -> absolute t index for a core with parity p."""
    A = [p, 2 + p, 1 - p, 3 - p]
    perm = np.empty(t_full, np.int64)
    for tb in range(t_full // 512):
        for sub in range(4):
            a = 128 * (4 * tb + A[sub])
            s = 512 * tb + 128 * sub
            perm[s:s + 128] = np.arange(a, a + 128)
    return perm


def combine_outputs(parts, t_full=T):
    """parts: list of 8 arrays [H+1, t_full] -> full output [B, t_full, H]."""
    out = np.empty((B, t_full, H), np.float32)
    for b in range(B):
        acc = np.zeros((H + 1, t_full), np.float32)
        for p in (0, 1):
            perm = _col_perm(p, t_full)
            acc[:, perm] += parts[2 * b + p]
        out[b] = (acc[:H, :] / acc[H:H + 1, :]).T
    return out


_NC_CACHE = {}


def kernel(x, Wk, bk, Wq, bq, Wv, bv):
    x = np.asarray(x, np.float32)
    t_full = x.shape[1]
    if t_full not in _NC_CACHE:
        _NC_CACHE[t_full] = build_kernel(t_full)
    nc = _NC_CACHE[t_full]
    ins = make_core_inputs(x, Wk, bk, Wq, bq, Wv, bv, t_full)
    res = run_bass_kernel_spmd(nc, ins, list(range(N_CORES)))
    parts = [res.results[i]["out_part"] for i in range(N_CORES)]
    return combine_outputs(parts, t_full)


if __name__ == "__main__":
    rng = np.random.default_rng(0)
    x = rng.standard_normal((B, T, C), dtype=np.float32)
    Wk = rng.standard_normal((C, H), dtype=np.float32) * SCALE
    Wq = rng.standard_normal((C, H), dtype=np.float32) * SCALE
    Wv = rng.standard_normal((C, H), dtype=np.float32) * SCALE
    bk = rng.standard_normal(H).astype(np.float32) * 0.02
    bq = rng.standard_normal(H).astype(np.float32) * 0.02
    bv = rng.standard_normal(H).astype(np.float32) * 0.02
    out = kernel(x=x, Wk=Wk, bk=bk, Wq=Wq, bq=bq, Wv=Wv, bv=bv)
    print(out.shape, out.dtype)
